# revision 6
# baseline (speedup 1.0000x reference)
"""Trainium2 Bass kernel for 3-hop GCN (nn_GNN_70978629534135).

Strategy (8 NeuronCores, SPMD):
  - Nodes are permuted by in-degree (descending) and snake-dealt across the 8
    cores so every core owns NL=12544 slots (12500 real + 44 zero "fake" pads)
    with a near-identical degree profile.
  - All edge indices are remapped into the permuted id space on the host, and
    partitioned by destination core.  For each 128-node tile the in-edges are
    padded to the tile-max degree K_t and stored as a [128, K_t] int32 gather
    table (pad slots point at a row that is always zero).
  - Since matmul distributes over the neighbor sum, each hop gathers raw h
    (not h@W): indirect-DMA gather of 512B rows from the allgathered h table,
    DVE tree-sum over the K slots, PE transpose, matmul (+bias as a rank-1
    PSUM accumulation), ReLU.
  - 3 AllGathers total (encoder out, hop1 out, hop2 out); hop3 feeds the
    decoder + log_softmax locally.

kernel(**inputs) takes the FULL unsharded inputs and returns the FULL output.
"""

import os
import numpy as np

N, E, F, H, D, R, C, W = 100_000, 600_000, 128, 256, 128, 3, 40, 8
P = 128


# ---------------------------------------------------------------- host prep
def _build_plan(edge_index, n, w, nl):
    tiles = nl // P
    src = edge_index[0].astype(np.int64)
    dst = edge_index[1].astype(np.int64)
    deg = np.bincount(dst, minlength=n)
    order = np.argsort(-deg, kind="stable")
    new_of_old = np.empty(n, dtype=np.int64)
    pos = np.arange(n)
    new_of_old[order] = (pos % w) * nl + (pos // w)
    src_n = new_of_old[src]
    dst_n = new_of_old[dst]
    zrow = nl - 1  # core 0's last slot: fake (zero) as long as n <= w*(nl-1)+...

    per_core = []
    Ks = np.zeros(tiles, dtype=np.int64)
    csr = []
    for c in range(w):
        m = (dst_n >= c * nl) & (dst_n < (c + 1) * nl)
        d_loc = dst_n[m] - c * nl
        s_glob = src_n[m]
        o = np.argsort(d_loc, kind="stable")
        d_loc, s_glob = d_loc[o], s_glob[o]
        counts = np.bincount(d_loc, minlength=nl)
        rowptr = np.concatenate([[0], np.cumsum(counts)])
        csr.append((d_loc, s_glob, rowptr))
        np.maximum(Ks, counts.reshape(tiles, P).max(axis=1), out=Ks)

    offs = np.concatenate([[0], np.cumsum(Ks)]).astype(np.int64)
    sumk = int(Ks.sum())
    for c in range(w):
        d_loc, s_glob, rowptr = csr[c]
        rank = np.arange(len(d_loc)) - rowptr[d_loc]
        col = offs[d_loc // P] + rank
        idx = np.full((P, sumk), zrow, dtype=np.int32)
        idx[d_loc % P, col] = s_glob
        per_core.append(idx)

    old_of_new = np.full(w * nl, -1, dtype=np.int64)
    old_of_new[new_of_old] = np.arange(n)
    return new_of_old, old_of_new, per_core, Ks.astype(int), offs, sumk


# ------------------------------------------------------------- device program
def _emit(tc, io, cfg):
    import concourse.bass as bass
    from concourse import mybir
    from concourse.masks import make_identity

    nc = tc.nc
    f32 = mybir.dt.float32
    nl, tiles, sumk = cfg["NL"], cfg["TILES"], cfg["SUMK"]
    Ks, offs = cfg["Ks"], cfg["offs"]
    n_fake = nl - cfg["REAL_PER_CORE"]  # fake slots at the tail of last tile
    h2, r_hops = cfg["H"], cfg["R"]
    ncls = cfg["C"]
    AG_GROUPS = [list(range(cfg["W"]))]

    def rank1(psum_ap, ones, bias_ap, stop=True):
        nc.tensor.matmul(psum_ap, lhsT=ones, rhs=bias_ap, start=False, stop=stop)

    with tc.tile_pool(name="const", bufs=1) as cp, \
         tc.tile_pool(name="sb", bufs=3) as sb, \
         tc.tile_pool(name="sb_g", bufs=3) as sbg, \
         tc.tile_pool(name="sb_h", bufs=3) as sbh, \
         tc.tile_pool(name="ps", bufs=2, space="PSUM") as ps:

        def ptile(tag, width):
            t = ps.tile([P, width], mybir.dt.float32, space="PSUM",
                        tag=tag, name=tag)
            return t

        ident = cp.tile([P, P], f32)
        make_identity(nc, ident[:])
        zeros128 = cp.tile([P, P], f32)
        nc.gpsimd.memset(zeros128[:], 0.0)
        real_end = cfg["REAL_PER_CORE"]

        def store_h(dst_dram, t, h_tile):
            lo, hi = t * P, (t + 1) * P
            real_hi = min(hi, real_end)
            if real_hi > lo:
                nc.sync.dma_start(dst_dram[lo:real_hi, :], h_tile[0:real_hi - lo, :])
            if hi > real_hi:
                nc.sync.dma_start(dst_dram[real_hi:hi, :],
                                  zeros128[0:hi - real_hi, :])
        ones = cp.tile([1, P], f32)
        nc.gpsimd.memset(ones[:], 1.0)

        # resident weights
        w1 = cp.tile([P, h2], f32)          # enc_w1 [F, H]
        nc.sync.dma_start(w1[:], io["enc_w1"][:])
        b1 = cp.tile([1, h2], f32)
        nc.sync.dma_start(b1[:], io["enc_b1"][:])
        w2 = cp.tile([P, h2], f32)          # enc_w2 [H, D] -> [:, j*128:] = rows j
        for j in range(h2 // P):
            nc.sync.dma_start(w2[:, j * P:(j + 1) * P], io["enc_w2"][j * P:(j + 1) * P, :])
        b2 = cp.tile([1, P], f32)
        nc.sync.dma_start(b2[:], io["enc_b2"][:])
        gw = cp.tile([P, r_hops * P], f32)  # gcn_w stacked [R*D, D]
        for r in range(r_hops):
            nc.sync.dma_start(gw[:, r * P:(r + 1) * P], io["gcn_w"][r * P:(r + 1) * P, :])
        gb = cp.tile([1, r_hops * P], f32)
        nc.sync.dma_start(gb[:], io["gcn_b"][:])
        dw1 = cp.tile([P, P], f32)
        nc.sync.dma_start(dw1[:], io["dec_w1"][:])
        db1 = cp.tile([1, P], f32)
        nc.sync.dma_start(db1[:], io["dec_b1"][:])
        dw2 = cp.tile([P, ncls], f32)
        nc.sync.dma_start(dw2[:], io["dec_w2"][:])
        db2 = cp.tile([1, ncls], f32)
        nc.sync.dma_start(db2[:], io["dec_b2"][:])

        idx_sb = cp.tile([P, sumk], mybir.dt.int32)
        nc.sync.dma_start(idx_sb[:], io["idx"][:])

        # internal DRAM: allgather bounce + full tables
        bounce = [nc.dram_tensor(f"bounce{r}", [nl, P], f32, kind="Internal")
                  for r in range(r_hops)]
        hfull = [nc.dram_tensor(f"hfull{r}", [cfg["W"] * nl, P], f32,
                                kind="Internal", addr_space="Shared")
                 for r in range(r_hops)]

        # ---------------- encoder ----------------
        for t in range(tiles):
            xT_t = sb.tile([P, P], f32, name="xT_t")
            nc.sync.dma_start(xT_t[:], io["xT"][:, t * P:(t + 1) * P])
            p1 = ptile("p1", h2)
            nc.tensor.matmul(p1[:], lhsT=xT_t[:], rhs=w1[:], start=True, stop=False)
            rank1(p1[:], ones[:], b1[:])
            z1 = sbh.tile([P, h2], f32, name="z1")
            nc.scalar.activation(z1[:], p1[:], mybir.ActivationFunctionType.Relu)
            z1T = sbh.tile([P, h2], f32, name="z1T")
            for j in range(h2 // P):
                ptj = ptile("pt", P)
                nc.tensor.transpose(ptj[:], z1[:, j * P:(j + 1) * P], ident[:])
                nc.vector.tensor_copy(z1T[:, j * P:(j + 1) * P], ptj[:])
            p2 = ptile("pmm", P)
            for j in range(h2 // P):
                nc.tensor.matmul(p2[:], lhsT=z1T[:, j * P:(j + 1) * P],
                                 rhs=w2[:, j * P:(j + 1) * P],
                                 start=(j == 0), stop=False)
            rank1(p2[:], ones[:], b2[:])
            h0 = sbh.tile([P, P], f32, name="h0")
            nc.scalar.activation(h0[:], p2[:], mybir.ActivationFunctionType.Copy)
            store_h(bounce[0], t, h0)

        nc.gpsimd.collective_compute(
            "AllGather", mybir.AluOpType.bypass,
            ins=[bounce[0][:]], outs=[hfull[0][:]], replica_groups=AG_GROUPS)

        if "dbg_hf" in io:
            nc.sync.dma_start(io["dbg_hf"][:], hfull[0][:])

        # ---------------- hops 1..R-1 (write bounce, allgather) ----------
        def gather_sum(t, src_full):
            K = int(Ks[t])
            off = int(offs[t])
            g = sbg.tile([P, max(K, 1) * P], f32, name="g")
            if K == 0:
                nc.vector.memset(g[:, :P], 0.0)
                return g
            for k in range(K):
                nc.gpsimd.indirect_dma_start(
                    out=g[:, k * P:(k + 1) * P], out_offset=None, in_=src_full[:],
                    in_offset=bass.IndirectOffsetOnAxis(
                        ap=idx_sb[:, off + k:off + k + 1], axis=0))
            kk = K
            while kk > 1:
                if kk % 2:
                    nc.vector.tensor_tensor(
                        g[:, :P], g[:, :P], g[:, (kk - 1) * P:kk * P],
                        op=mybir.AluOpType.add)
                    kk -= 1
                half = kk // 2
                nc.vector.tensor_tensor(
                    g[:, :half * P], g[:, :half * P], g[:, half * P:kk * P],
                    op=mybir.AluOpType.add)
                kk = half
            return g

        def transpose_sb(src_ap, name):
            pt = ptile("pt", P)
            nc.tensor.transpose(pt[:], src_ap, ident[:])
            out = sbh.tile([P, P], f32, name=f"sb_{name}")
            nc.vector.tensor_copy(out[:], pt[:])
            return out

        for r in range(r_hops - 1):
            for t in range(tiles):
                g = gather_sum(t, hfull[r])
                sT = transpose_sb(g[:, :P], f"s{r}")
                ph = ptile("pmm", P)
                nc.tensor.matmul(ph[:], lhsT=sT[:], rhs=gw[:, r * P:(r + 1) * P],
                                 start=True, stop=False)
                rank1(ph[:], ones[:], gb[:, r * P:(r + 1) * P])
                hn = sbh.tile([P, P], f32, name="hn")
                nc.scalar.activation(hn[:], ph[:], mybir.ActivationFunctionType.Relu)
                store_h(bounce[r + 1], t, hn)
            nc.gpsimd.collective_compute(
                "AllGather", mybir.AluOpType.bypass,
                ins=[bounce[r + 1][:]], outs=[hfull[r + 1][:]],
                replica_groups=AG_GROUPS)

        # ---------------- hop R + decoder + log_softmax ------------------
        rl = r_hops - 1
        for t in range(tiles):
            g = gather_sum(t, hfull[rl])
            sT = transpose_sb(g[:, :P], "s_last")
            ph3 = ptile("pmm", P)
            nc.tensor.matmul(ph3[:], lhsT=sT[:], rhs=gw[:, rl * P:(rl + 1) * P],
                             start=True, stop=False)
            rank1(ph3[:], ones[:], gb[:, rl * P:(rl + 1) * P])
            h3 = sbh.tile([P, P], f32, name="h3")
            nc.scalar.activation(h3[:], ph3[:], mybir.ActivationFunctionType.Relu)

            h3T = transpose_sb(h3[:], "h3")
            pz = ptile("pmm", P)
            nc.tensor.matmul(pz[:], lhsT=h3T[:], rhs=dw1[:], start=True, stop=False)
            rank1(pz[:], ones[:], db1[:])
            z = sbh.tile([P, P], f32, name="z")
            nc.scalar.activation(z[:], pz[:], mybir.ActivationFunctionType.Relu)

            zT = transpose_sb(z[:], "z")
            pl_t = ptile("pmm", P)
            pl = pl_t[:, 0:ncls]
            nc.tensor.matmul(pl[:], lhsT=zT[:], rhs=dw2[:], start=True, stop=False)
            rank1(pl[:], ones[:], db2[:])

            # log_softmax over the free axis (ncls)
            nmx = sb.tile([P, 1], f32, name="nmx")
            nc.vector.tensor_reduce(nmx[:], pl[:], axis=mybir.AxisListType.X,
                                    op=mybir.AluOpType.max, negate=True)
            ex = sb.tile([P, ncls], f32, name="ex")
            esum = sb.tile([P, 1], f32, name="esum")
            nc.scalar.activation(ex[:], pl[:], mybir.ActivationFunctionType.Exp,
                                 bias=nmx[:, 0:1], accum_out=esum[:, 0:1])
            lg = sb.tile([P, 1], f32, name="lg")
            nc.scalar.activation(lg[:], esum[:], mybir.ActivationFunctionType.Ln)
            shift = sb.tile([P, 1], f32, name="shift")
            nc.vector.tensor_tensor(shift[:], nmx[:], lg[:],
                                    op=mybir.AluOpType.subtract)  # -max - ln(sum)
            ot = sb.tile([P, ncls], f32, name="ot")
            nc.vector.tensor_tensor(ot[:], pl[:],
                                    shift[:, 0:1].to_broadcast([P, ncls]),
                                    op=mybir.AluOpType.add)
            nc.sync.dma_start(io["out"][t * P:(t + 1) * P, :], ot[:])


def _build_program(cfg):
    from concourse import bacc, mybir, tile

    f32 = mybir.dt.float32
    i32 = mybir.dt.int32
    nc = bacc.Bacc("TRN2", target_bir_lowering=False, debug=False,
                   num_devices=cfg["W"])
    nl, sumk, h2, ncls, r_hops = cfg["NL"], cfg["SUMK"], cfg["H"], cfg["C"], cfg["R"]
    io = {
        "xT": nc.dram_tensor("xT", [P, nl], f32, kind="ExternalInput").ap(),
        "idx": nc.dram_tensor("idx", [P, sumk], i32, kind="ExternalInput").ap(),
        "enc_w1": nc.dram_tensor("enc_w1", [P, h2], f32, kind="ExternalInput").ap(),
        "enc_b1": nc.dram_tensor("enc_b1", [1, h2], f32, kind="ExternalInput").ap(),
        "enc_w2": nc.dram_tensor("enc_w2", [h2, P], f32, kind="ExternalInput").ap(),
        "enc_b2": nc.dram_tensor("enc_b2", [1, P], f32, kind="ExternalInput").ap(),
        "gcn_w": nc.dram_tensor("gcn_w", [r_hops * P, P], f32, kind="ExternalInput").ap(),
        "gcn_b": nc.dram_tensor("gcn_b", [1, r_hops * P], f32, kind="ExternalInput").ap(),
        "dec_w1": nc.dram_tensor("dec_w1", [P, P], f32, kind="ExternalInput").ap(),
        "dec_b1": nc.dram_tensor("dec_b1", [1, P], f32, kind="ExternalInput").ap(),
        "dec_w2": nc.dram_tensor("dec_w2", [P, ncls], f32, kind="ExternalInput").ap(),
        "dec_b2": nc.dram_tensor("dec_b2", [1, ncls], f32, kind="ExternalInput").ap(),
        "out": nc.dram_tensor("out", [nl, ncls], f32, kind="ExternalOutput").ap(),
    }
    if cfg.get("DEBUG"):
        io["dbg_hf"] = nc.dram_tensor(
            "dbg_hf", [cfg["W"] * nl, P], f32, kind="ExternalOutput").ap()
    with tile.TileContext(nc) as tc:
        _emit(tc, io, cfg)
    nc.compile()
    return nc


_CACHE = {}
LAST_RESULT = None


def _make_cfg(Ks, offs, sumk, nl, w, real_per_core):
    return dict(NL=nl, TILES=nl // P, SUMK=sumk, Ks=Ks, offs=offs,
                H=H, R=R, C=C, W=w, REAL_PER_CORE=real_per_core)


def kernel(x, edge_index, enc_w1, enc_b1, enc_w2, enc_b2,
           gcn_w, gcn_b, dec_w1, dec_b1, dec_w2, dec_b2):
    global LAST_RESULT
    from concourse.bass_utils import run_bass_kernel_spmd

    nl = 12544
    x = np.asarray(x, dtype=np.float32)
    edge_index = np.asarray(edge_index)
    new_of_old, old_of_new, per_core_idx, Ks, offs, sumk = _build_plan(
        edge_index, N, W, nl)

    key = ("prog", sumk, tuple(Ks.tolist()))
    if key not in _CACHE:
        cfg = _make_cfg(Ks, offs, sumk, nl, W, N // W)
        _CACHE[key] = (_build_program(cfg), cfg)
    nc, cfg = _CACHE[key]

    # per-core inputs
    weights = {
        "enc_w1": np.asarray(enc_w1, np.float32),
        "enc_b1": np.asarray(enc_b1, np.float32).reshape(1, H),
        "enc_w2": np.asarray(enc_w2, np.float32),
        "enc_b2": np.asarray(enc_b2, np.float32).reshape(1, D),
        "gcn_w": np.asarray(gcn_w, np.float32).reshape(R * D, D),
        "gcn_b": np.asarray(gcn_b, np.float32).reshape(1, R * D),
        "dec_w1": np.asarray(dec_w1, np.float32),
        "dec_b1": np.asarray(dec_b1, np.float32).reshape(1, D),
        "dec_w2": np.asarray(dec_w2, np.float32),
        "dec_b2": np.asarray(dec_b2, np.float32).reshape(1, C),
    }
    in_maps = []
    for c in range(W):
        ids = old_of_new[c * nl:(c + 1) * nl]
        xs = np.zeros((nl, F), np.float32)
        real = ids >= 0
        xs[real] = x[ids[real]]
        im = dict(weights)
        im["xT"] = np.ascontiguousarray(xs.T)
        im["idx"] = per_core_idx[c]
        in_maps.append(im)

    res = run_bass_kernel_spmd(
        nc, in_maps, core_ids=list(range(W)),
        trace=bool(int(os.environ.get("KERNEL_TRACE", "0"))))
    LAST_RESULT = res

    out = np.empty((N, C), np.float32)
    for c in range(W):
        ids = old_of_new[c * nl:(c + 1) * nl]
        real = ids >= 0
        out[ids[real]] = np.asarray(res.results[c]["out"])[real]
    return out


# revision 9
# speedup vs baseline: 1.1862x; 1.1862x over previous
"""Trainium2 Bass kernel for 3-hop GCN (nn_GNN_70978629534135).

Strategy (8 NeuronCores, SPMD):
  - Nodes are permuted by in-degree (descending) and snake-dealt across the 8
    cores so every core owns NL=12544 slots (12500 real + 44 zero "fake" pads)
    with a near-identical degree profile.
  - All edge indices are remapped into the permuted id space on the host, and
    partitioned by destination core.  For each 128-node tile the in-edges are
    padded to the tile-max degree K_t and stored as a [128, K_t] int32 gather
    table (pad slots point at a row that is always zero).
  - Since matmul distributes over the neighbor sum, each hop gathers raw h
    (not h@W): indirect-DMA gather of 512B rows from the allgathered h table,
    DVE tree-sum over the K slots, PE transpose, matmul (+bias as a rank-1
    PSUM accumulation), ReLU.
  - 3 AllGathers total (encoder out, hop1 out, hop2 out); hop3 feeds the
    decoder + log_softmax locally.

kernel(**inputs) takes the FULL unsharded inputs and returns the FULL output.
"""

import os
import numpy as np

N, E, F, H, D, R, C, W = 100_000, 600_000, 128, 256, 128, 3, 40, 8
P = 128


# ---------------------------------------------------------------- host prep
def _build_plan(edge_index, n, w, nl):
    tiles = nl // P
    src = edge_index[0].astype(np.int64)
    dst = edge_index[1].astype(np.int64)
    deg = np.bincount(dst, minlength=n)
    order = np.argsort(-deg, kind="stable")
    new_of_old = np.empty(n, dtype=np.int64)
    pos = np.arange(n)
    new_of_old[order] = (pos % w) * nl + (pos // w)
    src_n = new_of_old[src]
    dst_n = new_of_old[dst]
    zrow = nl - 1  # core 0's last slot: fake (zero) as long as n <= w*(nl-1)+...

    per_core = []
    Ks = np.zeros(tiles, dtype=np.int64)
    csr = []
    for c in range(w):
        m = (dst_n >= c * nl) & (dst_n < (c + 1) * nl)
        d_loc = dst_n[m] - c * nl
        s_glob = src_n[m]
        o = np.argsort(d_loc, kind="stable")
        d_loc, s_glob = d_loc[o], s_glob[o]
        counts = np.bincount(d_loc, minlength=nl)
        rowptr = np.concatenate([[0], np.cumsum(counts)])
        csr.append((d_loc, s_glob, rowptr))
        np.maximum(Ks, counts.reshape(tiles, P).max(axis=1), out=Ks)

    offs = np.concatenate([[0], np.cumsum(Ks)]).astype(np.int64)
    sumk = int(Ks.sum())
    for c in range(w):
        d_loc, s_glob, rowptr = csr[c]
        rank = np.arange(len(d_loc)) - rowptr[d_loc]
        col = offs[d_loc // P] + rank
        idx = np.full((P, sumk), zrow, dtype=np.int32)
        idx[d_loc % P, col] = s_glob
        per_core.append(idx)

    old_of_new = np.full(w * nl, -1, dtype=np.int64)
    old_of_new[new_of_old] = np.arange(n)
    return new_of_old, old_of_new, per_core, Ks.astype(int), offs, sumk


# ------------------------------------------------------------- device program
def _emit(tc, io, cfg):
    import concourse.bass as bass
    from concourse import mybir
    from concourse.masks import make_identity

    nc = tc.nc
    f32 = mybir.dt.float32
    bf16 = mybir.dt.bfloat16
    nl, tiles, sumk = cfg["NL"], cfg["TILES"], cfg["SUMK"]
    Ks, offs = cfg["Ks"], cfg["offs"]
    n_fake = nl - cfg["REAL_PER_CORE"]  # fake slots at the tail of last tile
    h2, r_hops = cfg["H"], cfg["R"]
    ncls = cfg["C"]
    AG_GROUPS = [list(range(cfg["W"]))]

    def rank1(psum_ap, ones, bias_ap, stop=True):
        nc.tensor.matmul(psum_ap, lhsT=ones, rhs=bias_ap, start=False, stop=stop)

    with tc.tile_pool(name="const", bufs=1) as cp, \
         tc.tile_pool(name="sb", bufs=3) as sb, \
         tc.tile_pool(name="sb_g", bufs=3) as sbg, \
         tc.tile_pool(name="sb_h", bufs=3) as sbh, \
         tc.tile_pool(name="ps", bufs=2, space="PSUM") as ps:

        def ptile(tag, width, dtype=None):
            t = ps.tile([P, width], dtype or mybir.dt.float32, space="PSUM",
                        tag=tag, name=tag)
            return t

        ident = cp.tile([P, P], f32)
        make_identity(nc, ident[:])
        ident_bf = cp.tile([P, P], bf16)
        make_identity(nc, ident_bf[:])
        zeros128 = cp.tile([P, P], bf16)
        nc.gpsimd.memset(zeros128[:], 0.0)
        real_end = cfg["REAL_PER_CORE"]

        def store_h(dst_dram, t, h_tile):
            lo, hi = t * P, (t + 1) * P
            real_hi = min(hi, real_end)
            if real_hi > lo:
                nc.sync.dma_start(dst_dram[lo:real_hi, :], h_tile[0:real_hi - lo, :])
            if hi > real_hi:
                nc.sync.dma_start(dst_dram[real_hi:hi, :],
                                  zeros128[0:hi - real_hi, :])
        ones = cp.tile([1, P], bf16)
        nc.gpsimd.memset(ones[:], 1.0)

        # resident weights
        w1 = cp.tile([P, h2], bf16)          # enc_w1 [F, H]
        nc.sync.dma_start(w1[:], io["enc_w1"][:])
        b1 = cp.tile([1, h2], bf16)
        nc.sync.dma_start(b1[:], io["enc_b1"][:])
        w2 = cp.tile([P, h2], bf16)          # enc_w2 [H, D] -> [:, j*128:] = rows j
        for j in range(h2 // P):
            nc.sync.dma_start(w2[:, j * P:(j + 1) * P], io["enc_w2"][j * P:(j + 1) * P, :])
        b2 = cp.tile([1, P], bf16)
        nc.sync.dma_start(b2[:], io["enc_b2"][:])
        gw = cp.tile([P, r_hops * P], bf16)  # gcn_w stacked [R*D, D]
        for r in range(r_hops):
            nc.sync.dma_start(gw[:, r * P:(r + 1) * P], io["gcn_w"][r * P:(r + 1) * P, :])
        gb = cp.tile([1, r_hops * P], bf16)
        nc.sync.dma_start(gb[:], io["gcn_b"][:])
        dw1 = cp.tile([P, P], bf16)
        nc.sync.dma_start(dw1[:], io["dec_w1"][:])
        db1 = cp.tile([1, P], bf16)
        nc.sync.dma_start(db1[:], io["dec_b1"][:])
        dw2 = cp.tile([P, ncls], bf16)
        nc.sync.dma_start(dw2[:], io["dec_w2"][:])
        db2 = cp.tile([1, ncls], bf16)
        nc.sync.dma_start(db2[:], io["dec_b2"][:])

        idx_sb = cp.tile([P, sumk], mybir.dt.int32)
        nc.sync.dma_start(idx_sb[:], io["idx"][:])

        # internal DRAM: allgather bounce + full tables
        bounce = [nc.dram_tensor(f"bounce{r}", [nl, P], bf16, kind="Internal")
                  for r in range(r_hops)]
        hfull = [nc.dram_tensor(f"hfull{r}", [cfg["W"] * nl, P], bf16,
                                kind="Internal", addr_space="Shared")
                 for r in range(r_hops)]

        # ---------------- encoder ----------------
        for t in range(tiles):
            xT_t = sb.tile([P, P], bf16, name="xT_t")
            nc.sync.dma_start(xT_t[:], io["xT"][:, t * P:(t + 1) * P])
            p1 = ptile("p1", h2)
            nc.tensor.matmul(p1[:], lhsT=xT_t[:], rhs=w1[:], start=True, stop=False)
            rank1(p1[:], ones[:], b1[:])
            z1 = sbh.tile([P, h2], f32, name="z1")
            nc.scalar.activation(z1[:], p1[:], mybir.ActivationFunctionType.Relu)
            z1T = sbh.tile([P, h2], bf16, name="z1T")
            for j in range(h2 // P):
                ptj = ptile("pt", P)
                nc.tensor.transpose(ptj[:], z1[:, j * P:(j + 1) * P], ident[:])
                nc.vector.tensor_copy(z1T[:, j * P:(j + 1) * P], ptj[:])
            p2 = ptile("pmm", P)
            for j in range(h2 // P):
                nc.tensor.matmul(p2[:], lhsT=z1T[:, j * P:(j + 1) * P],
                                 rhs=w2[:, j * P:(j + 1) * P],
                                 start=(j == 0), stop=False)
            rank1(p2[:], ones[:], b2[:])
            h0 = sbh.tile([P, P], bf16, name="h0")
            nc.scalar.activation(h0[:], p2[:], mybir.ActivationFunctionType.Copy)
            store_h(bounce[0], t, h0)

        nc.gpsimd.collective_compute(
            "AllGather", mybir.AluOpType.bypass,
            ins=[bounce[0][:]], outs=[hfull[0][:]], replica_groups=AG_GROUPS)

        if "dbg_hf" in io:
            nc.sync.dma_start(io["dbg_hf"][:], hfull[0][:])

        # ---------------- hops 1..R-1 (write bounce, allgather) ----------
        def gather_sum(t, src_full):
            K = int(Ks[t])
            off = int(offs[t])
            g = sbg.tile([P, max(K, 1) * P], bf16, name="g")
            sf = sbg.tile([P, max((K + 1) // 2, 1) * P], f32, name="sf")
            if K == 0:
                nc.vector.memset(sf[:, :P], 0.0)
                return sf
            for k in range(K):
                nc.gpsimd.indirect_dma_start(
                    out=g[:, k * P:(k + 1) * P], out_offset=None, in_=src_full[:],
                    in_offset=bass.IndirectOffsetOnAxis(
                        ap=idx_sb[:, off + k:off + k + 1], axis=0))
            # level 1: bf16 pairs -> f32
            for j in range(K // 2):
                nc.vector.tensor_tensor(
                    sf[:, j * P:(j + 1) * P], g[:, 2 * j * P:(2 * j + 1) * P],
                    g[:, (2 * j + 1) * P:(2 * j + 2) * P], op=mybir.AluOpType.add)
            if K % 2:
                nc.vector.tensor_copy(sf[:, (K // 2) * P:(K // 2 + 1) * P],
                                      g[:, (K - 1) * P:K * P])
            kk = (K + 1) // 2
            while kk > 1:
                if kk % 2:
                    nc.vector.tensor_tensor(
                        sf[:, :P], sf[:, :P], sf[:, (kk - 1) * P:kk * P],
                        op=mybir.AluOpType.add)
                    kk -= 1
                half = kk // 2
                nc.vector.tensor_tensor(
                    sf[:, :half * P], sf[:, :half * P], sf[:, half * P:kk * P],
                    op=mybir.AluOpType.add)
                kk = half
            return sf

        def transpose_sb(src_ap, name):
            idn = ident if src_ap.dtype == f32 else ident_bf
            pt = ptile("pt", P, dtype=src_ap.dtype)
            nc.tensor.transpose(pt[:], src_ap, idn[:])
            out = sbh.tile([P, P], bf16, name=f"sb_{name}")
            nc.vector.tensor_copy(out[:], pt[:])
            return out

        for r in range(r_hops - 1):
            for t in range(tiles):
                g = gather_sum(t, hfull[r])
                sT = transpose_sb(g[:, :P], f"s{r}")
                ph = ptile("pmm", P)
                nc.tensor.matmul(ph[:], lhsT=sT[:], rhs=gw[:, r * P:(r + 1) * P],
                                 start=True, stop=False)
                rank1(ph[:], ones[:], gb[:, r * P:(r + 1) * P])
                hn = sbh.tile([P, P], bf16, name="hn")
                nc.scalar.activation(hn[:], ph[:], mybir.ActivationFunctionType.Relu)
                store_h(bounce[r + 1], t, hn)
            nc.gpsimd.collective_compute(
                "AllGather", mybir.AluOpType.bypass,
                ins=[bounce[r + 1][:]], outs=[hfull[r + 1][:]],
                replica_groups=AG_GROUPS)

        # ---------------- hop R + decoder + log_softmax ------------------
        rl = r_hops - 1
        for t in range(tiles):
            g = gather_sum(t, hfull[rl])
            sT = transpose_sb(g[:, :P], "s_last")
            ph3 = ptile("pmm", P)
            nc.tensor.matmul(ph3[:], lhsT=sT[:], rhs=gw[:, rl * P:(rl + 1) * P],
                             start=True, stop=False)
            rank1(ph3[:], ones[:], gb[:, rl * P:(rl + 1) * P])
            h3 = sbh.tile([P, P], bf16, name="h3")
            nc.scalar.activation(h3[:], ph3[:], mybir.ActivationFunctionType.Relu)

            h3T = transpose_sb(h3[:], "h3")
            pz = ptile("pmm", P)
            nc.tensor.matmul(pz[:], lhsT=h3T[:], rhs=dw1[:], start=True, stop=False)
            rank1(pz[:], ones[:], db1[:])
            z = sbh.tile([P, P], bf16, name="z")
            nc.scalar.activation(z[:], pz[:], mybir.ActivationFunctionType.Relu)

            zT = transpose_sb(z[:], "z")
            pl_t = ptile("pmm", P)
            pl = pl_t[:, 0:ncls]
            nc.tensor.matmul(pl[:], lhsT=zT[:], rhs=dw2[:], start=True, stop=False)
            rank1(pl[:], ones[:], db2[:])

            # log_softmax over the free axis (ncls)
            nmx = sb.tile([P, 1], f32, name="nmx")
            nc.vector.tensor_reduce(nmx[:], pl[:], axis=mybir.AxisListType.X,
                                    op=mybir.AluOpType.max, negate=True)
            ex = sb.tile([P, ncls], f32, name="ex")
            esum = sb.tile([P, 1], f32, name="esum")
            nc.scalar.activation(ex[:], pl[:], mybir.ActivationFunctionType.Exp,
                                 bias=nmx[:, 0:1], accum_out=esum[:, 0:1])
            lg = sb.tile([P, 1], f32, name="lg")
            nc.scalar.activation(lg[:], esum[:], mybir.ActivationFunctionType.Ln)
            shift = sb.tile([P, 1], f32, name="shift")
            nc.vector.tensor_tensor(shift[:], nmx[:], lg[:],
                                    op=mybir.AluOpType.subtract)  # -max - ln(sum)
            ot = sb.tile([P, ncls], f32, name="ot")
            nc.vector.tensor_tensor(ot[:], pl[:],
                                    shift[:, 0:1].to_broadcast([P, ncls]),
                                    op=mybir.AluOpType.add)
            nc.sync.dma_start(io["out"][t * P:(t + 1) * P, :], ot[:])


def _build_program(cfg):
    from concourse import bacc, mybir, tile

    f32 = mybir.dt.float32
    bf16 = mybir.dt.bfloat16
    i32 = mybir.dt.int32
    nc = bacc.Bacc("TRN2", target_bir_lowering=False, debug=False,
                   num_devices=cfg["W"])
    nl, sumk, h2, ncls, r_hops = cfg["NL"], cfg["SUMK"], cfg["H"], cfg["C"], cfg["R"]
    io = {
        "xT": nc.dram_tensor("xT", [P, nl], bf16, kind="ExternalInput").ap(),
        "idx": nc.dram_tensor("idx", [P, sumk], i32, kind="ExternalInput").ap(),
        "enc_w1": nc.dram_tensor("enc_w1", [P, h2], bf16, kind="ExternalInput").ap(),
        "enc_b1": nc.dram_tensor("enc_b1", [1, h2], bf16, kind="ExternalInput").ap(),
        "enc_w2": nc.dram_tensor("enc_w2", [h2, P], bf16, kind="ExternalInput").ap(),
        "enc_b2": nc.dram_tensor("enc_b2", [1, P], bf16, kind="ExternalInput").ap(),
        "gcn_w": nc.dram_tensor("gcn_w", [r_hops * P, P], bf16, kind="ExternalInput").ap(),
        "gcn_b": nc.dram_tensor("gcn_b", [1, r_hops * P], bf16, kind="ExternalInput").ap(),
        "dec_w1": nc.dram_tensor("dec_w1", [P, P], bf16, kind="ExternalInput").ap(),
        "dec_b1": nc.dram_tensor("dec_b1", [1, P], bf16, kind="ExternalInput").ap(),
        "dec_w2": nc.dram_tensor("dec_w2", [P, ncls], bf16, kind="ExternalInput").ap(),
        "dec_b2": nc.dram_tensor("dec_b2", [1, ncls], bf16, kind="ExternalInput").ap(),
        "out": nc.dram_tensor("out", [nl, ncls], f32, kind="ExternalOutput").ap(),
    }
    if cfg.get("DEBUG"):
        io["dbg_hf"] = nc.dram_tensor(
            "dbg_hf", [cfg["W"] * nl, P], f32, kind="ExternalOutput").ap()
    with tile.TileContext(nc) as tc:
        _emit(tc, io, cfg)
    nc.compile()
    return nc


_CACHE = {}
LAST_RESULT = None


def _make_cfg(Ks, offs, sumk, nl, w, real_per_core):
    return dict(NL=nl, TILES=nl // P, SUMK=sumk, Ks=Ks, offs=offs,
                H=H, R=R, C=C, W=w, REAL_PER_CORE=real_per_core)


def kernel(x, edge_index, enc_w1, enc_b1, enc_w2, enc_b2,
           gcn_w, gcn_b, dec_w1, dec_b1, dec_w2, dec_b2):
    global LAST_RESULT
    from concourse.bass_utils import run_bass_kernel_spmd

    nl = 12544
    x = np.asarray(x, dtype=np.float32)
    edge_index = np.asarray(edge_index)
    new_of_old, old_of_new, per_core_idx, Ks, offs, sumk = _build_plan(
        edge_index, N, W, nl)

    key = ("prog", sumk, tuple(Ks.tolist()))
    if key not in _CACHE:
        cfg = _make_cfg(Ks, offs, sumk, nl, W, N // W)
        _CACHE[key] = (_build_program(cfg), cfg)
    nc, cfg = _CACHE[key]

    # per-core inputs
    import ml_dtypes
    bf = ml_dtypes.bfloat16
    weights = {
        "enc_w1": np.asarray(enc_w1, np.float32).astype(bf),
        "enc_b1": np.asarray(enc_b1, np.float32).reshape(1, H).astype(bf),
        "enc_w2": np.asarray(enc_w2, np.float32).astype(bf),
        "enc_b2": np.asarray(enc_b2, np.float32).reshape(1, D).astype(bf),
        "gcn_w": np.asarray(gcn_w, np.float32).reshape(R * D, D).astype(bf),
        "gcn_b": np.asarray(gcn_b, np.float32).reshape(1, R * D).astype(bf),
        "dec_w1": np.asarray(dec_w1, np.float32).astype(bf),
        "dec_b1": np.asarray(dec_b1, np.float32).reshape(1, D).astype(bf),
        "dec_w2": np.asarray(dec_w2, np.float32).astype(bf),
        "dec_b2": np.asarray(dec_b2, np.float32).reshape(1, C).astype(bf),
    }
    in_maps = []
    for c in range(W):
        ids = old_of_new[c * nl:(c + 1) * nl]
        xs = np.zeros((nl, F), np.float32)
        real = ids >= 0
        xs[real] = x[ids[real]]
        im = dict(weights)
        im["xT"] = np.ascontiguousarray(xs.T).astype(bf)
        im["idx"] = per_core_idx[c]
        in_maps.append(im)

    res = run_bass_kernel_spmd(
        nc, in_maps, core_ids=list(range(W)),
        trace=bool(int(os.environ.get("KERNEL_TRACE", "0"))))
    LAST_RESULT = res

    out = np.empty((N, C), np.float32)
    for c in range(W):
        ids = old_of_new[c * nl:(c + 1) * nl]
        real = ids >= 0
        out[ids[real]] = np.asarray(res.results[c]["out"])[real]
    return out
